# revision 36
# baseline (speedup 1.0000x reference)
"""LIAFResBlock forward on 8 Trainium2 NeuronCores (data-parallel over batch).

Self-contained: hardcodes shapes for x [16,64,8,56,56] -> out [16,128,8,28,28].

Math notes (vs the PyTorch/JAX reference):
  - conv biases are no-ops: every conv is followed by training-mode BN, which
    subtracts the per-channel mean, absorbing any per-channel constant.
  - the final mem_update on a binary {0,1} tensor is the identity because
    d = sigmoid(0.5) ~ 0.6225 and d*0.5 < 0.5, so out = lif_act(bn2(cv2)+bn_sc(sc)).
  - the first mem_update runs in "normalized" space: with a1 = g1*rstd1 (>0),
    v = m/a1 satisfies v[t] = d*v[t-1]*[v<=tau] + (cv1[t] + beta1/a1),
    spike[t] = v[t] > tau, tau = 0.5/a1. BN1 folds into a per-channel bias on
    cv1 plus a per-channel threshold.
  - BN batch stats are global over B=16: each core computes per-channel
    (sum, sumsq) partials; tiny AllReduces combine them.
  - shortcut threshold: scf <- -asc*scf + (0.5-bsc) on gpsimd during phase B;
    final out = 1[a2*cv2 + b2 > scf] elementwise.

Conv1 structure (phase A, per 392-px chunk = 15 streams instead of 18):
  x = xhi + xlo and W1 = Whi + Wlo (bf16 splits; products exact in fp32 PSUM).
  - 9 streams [Whi_k;Whi_k] (x) [xhi;xlo]    -> Whi_k*(xhi+xlo), all taps
  - 3 streams [Wlo(1,kw);Wlo(2,kw)] (x) [xhi;xhi_rowshift] -> row-adjacent
    tap pair of the lo-correction in ONE K=128 stream (the shifted copy is
    built on-chip: scalar copies the hi half, vector the shifted half)
  - 3 streams Wlo(0,kw) (x) xhi, K=64 (partial rows, oh>=1)
  The Wlo*xlo term (~2^-18 relative) is dropped.

Schedule notes (AllReduce latency ~8-12us + core launch skew ~17us):
  - AR1a: bn1+scn partial sums over t=0..5, hidden under conv1 t=6,7.
  - AR1b: bn1 remainder, issued the moment conv1 finishes; deferred t=6,7
    shortcut matmuls run inside its latency. Phase B blocks only on AR1b.
  - AR1c: scn remainder, lands early in phase B; scf threshold pass runs on
    the otherwise-idle gpsimd mid phase B.
  - AR2a: bn2 partial (t=0..5) hidden under conv2 t=6,7; AR2b (t=6,7) at
    phase B end is the only exposed tail latency.
  - epilogue: z' = a2*cv2+b2 on scalar (t<7) / gpsimd (t=7), compare on
    vector (t<7) / gpsimd (t=7), fp16 {0,1} output (host converts).

Implementation notes (hardware-measured):
  - conv2 fp16 single-pass (spikes are exact {0,1} in fp16; w2 fp16 rounding
    costs ~590 flips).
  - zero padding via partial-range matmuls (each off-center tap writes only
    its valid output sub-rectangle of PSUM), NOT zero-ringed input tiles
    (those force 224-byte DMA descriptors).
  - spikes are written as contiguous [128, 1568] fp16 on Vector (gpsimd's
    fp16 store path is ~16ns/elem and starves concurrent DVE SBUF access).
  - weights are transposed host-side so every DMA is contiguous.
"""
import math
import sys

import numpy as np

sys.path.insert(0, "/opt/trn_rl_repo")

import concourse.bass as bass  # noqa: E402
import concourse.bacc as bacc  # noqa: E402
import concourse.tile as tile  # noqa: E402
from concourse import mybir  # noqa: E402
from concourse.bass_utils import run_bass_kernel_spmd  # noqa: E402

dt = mybir.dt
Alu = mybir.AluOpType
Act = mybir.ActivationFunctionType

B, CIN, COUT, T, H, W = 16, 64, 128, 8, 56, 56
HO = WO = 28
NPIX = HO * WO          # 784
CHUNK = NPIX // 2       # 392 (one PSUM bank)
NCORES = 8
BPC = B // NCORES       # 2 samples per core
SPT = BPC * NPIX        # 1568 elements per fused (both-samples) timestep
NLOC = BPC * T * NPIX   # 12544 elements/channel per core
NGLOB = B * T * NPIX    # 100352 elements/channel globally
EPS = 1e-5
HW = H * W              # 3136 elements per unpadded input plane
TSPLIT = 6              # AR1 split: t < TSPLIT early, rest late
CNT_A = 4 * TSPLIT * CHUNK        # elems/channel/core in the early partial
CNT_B = 4 * (T - TSPLIT) * CHUNK  # and in the late partial
A2SPLIT = 5             # AR2 split: earlier, so AR2a clears the serial cc
CNT_2A = 4 * A2SPLIT * CHUNK      # stream well before AR2b triggers
CNT_2B = 4 * (T - A2SPLIT) * CHUNK


def _ap(base, off, free):
    """Sub-view of an SBUF AP: keep partition dim, custom free dims."""
    return bass.AP(tensor=base.tensor, offset=base.offset + off,
                   ap=[base.ap[0]] + free)


def build_nc(d: float) -> bass.Bass:
    nc = bacc.Bacc("TRN2", target_bir_lowering=False, num_devices=NCORES)

    xhi_d = nc.dram_tensor("xhi", [BPC, CIN, T, H, W], dt.bfloat16,
                           kind="ExternalInput")
    xlo_d = nc.dram_tensor("xlo", [BPC, CIN, T, H, W], dt.bfloat16,
                           kind="ExternalInput")
    # host-pretransposed lhsT weights
    w1h_d = nc.dram_tensor("w1h", [2 * CIN, 9, COUT], dt.bfloat16,
                           kind="ExternalInput")   # [Whi; Whi] per tap
    w1p_d = nc.dram_tensor("w1p", [2 * CIN, 3, COUT], dt.bfloat16,
                           kind="ExternalInput")   # [Wlo(1,kw); Wlo(2,kw)]
    w1z_d = nc.dram_tensor("w1z", [CIN, 3, COUT], dt.bfloat16,
                           kind="ExternalInput")   # Wlo(0,kw)
    w1c_d = nc.dram_tensor("w1c", [2 * CIN, COUT], dt.bfloat16,
                           kind="ExternalInput")   # [Wlo(0,0); Wlo(0,1)]
    wsh_d = nc.dram_tensor("wsh", [2 * CIN, COUT], dt.bfloat16,
                           kind="ExternalInput")
    wsl_d = nc.dram_tensor("wsl", [2 * CIN, COUT], dt.bfloat16,
                           kind="ExternalInput")
    w2t_d = nc.dram_tensor("w2t", [COUT, 9, COUT], dt.float16,
                           kind="ExternalInput")
    # columns: bn1_g, bn1_b, bn2_g, bn2_b, scn_g, scn_b
    pars_d = nc.dram_tensor("pars", [COUT, 6], dt.float32, kind="ExternalInput")
    out_d = nc.dram_tensor("out", [BPC, COUT, T, HO, WO], dt.uint8,
                           kind="ExternalOutput")

    from contextlib import ExitStack
    with tile.TileContext(nc) as tc, ExitStack() as stk:
        big = stk.enter_context(tc.tile_pool(name="big", bufs=1))
        const = stk.enter_context(tc.tile_pool(name="const", bufs=1))
        psum = stk.enter_context(tc.tile_pool(name="psum", bufs=8, space="PSUM"))
        dramp = stk.enter_context(tc.tile_pool(name="dramp", bufs=1, space="DRAM"))

        # warmup AllReduce first: starts the one-time cc-stream init (~60us)
        # as early as possible, fully overlapped with phase A
        wrm = const.tile([1, 1], dt.float32)
        nc.vector.memset(wrm[:, :], 0.0)
        ccwi = dramp.tile([1, 1], dt.float32)
        ccwo = dramp.tile([1, 1], dt.float32, addr_space="Shared")
        nc.sync.dma_start(out=ccwi[:, :], in_=wrm[:, :])
        nc.gpsimd.collective_compute(
            "AllReduce", Alu.add, replica_groups=[list(range(NCORES))],
            ins=[ccwi[:, :].opt()], outs=[ccwo[:, :].opt()])

        # ---- conv1 weights first (first matmul needs only w1h + x tile) ----
        w1h = const.tile([2 * CIN, 9, COUT], dt.bfloat16)
        nc.sync.dma_start(out=w1h[:, :, :], in_=w1h_d[:, :, :])
        w1p = const.tile([2 * CIN, 3, COUT], dt.bfloat16)
        nc.sync.dma_start(out=w1p[:, :, :], in_=w1p_d[:, :, :])
        w1z = const.tile([CIN, 3, COUT], dt.bfloat16)
        nc.sync.dma_start(out=w1z[:, :, :], in_=w1z_d[:, :, :])
        w1c = const.tile([2 * CIN, COUT], dt.bfloat16)
        nc.sync.dma_start(out=w1c[:, :], in_=w1c_d[:, :])
        wsh = const.tile([2 * CIN, COUT], dt.bfloat16)
        wsl = const.tile([2 * CIN, COUT], dt.bfloat16)
        eps_t = const.tile([COUT, 1], dt.float32)
        nc.vector.memset(eps_t[:, :], EPS)

        # ---- persistent activation buffers, (t, s)-major free layout ----
        cv1f = big.tile([COUT, NLOC], dt.float32)   # conv1 raw, then c'=cv1+btil
        scf = big.tile([COUT, NLOC], dt.float32)    # shortcut raw, then thr
        cv2f = big.tile([COUT, NLOC], dt.float32)   # conv2 raw
        st1 = const.tile([COUT, 4 * T, 6], dt.float32)
        sts = const.tile([COUT, 4 * T, 6], dt.float32)
        st2 = const.tile([COUT, 4 * T, 6], dt.float32)

        def sums_into(dst2, mv, cnt):
            """(mean,var)[128,2] -> (sum/NGLOB, sumsq/NGLOB) into dst2.

            Pre-scaled by 1/NGLOB so the AllReduced totals are directly the
            global mean and mean-square."""
            w = float(cnt) / NGLOB
            nc.vector.tensor_scalar_mul(dst2[:, 0:1], mv[:, 0:1], w)
            nc.vector.scalar_tensor_tensor(
                dst2[:, 1:2], mv[:, 0:1], w, mv[:, 0:1],
                Alu.mult, Alu.mult)
            nc.vector.scalar_tensor_tensor(
                dst2[:, 1:2], mv[:, 1:2], w, dst2[:, 1:2],
                Alu.mult, Alu.add)

        # ===== phase A: conv1 + shortcut (bf16 hi/lo, partial-range) ========
        # conv1 tap (kh,kw): input (2oh+kh-1, 2ow+kw-1); kh=0 needs oh>=1,
        # kw=0 needs ow>=1, all else full. Chunk c covers oh in [14c, 14c+14).
        def tap_geom(kh, kw, c):
            if kh == 0:
                oh0 = 1 if c == 0 else 14
                nr = 13 if c == 0 else 14
            else:
                oh0, nr = 14 * c, 14
            rbase = (2 * oh0 + kh - 1) * W
            if kw == 0:
                cbase, ncol, oc = 1, WO - 1, 1
            else:
                cbase, ncol, oc = kw - 1, WO, 0
            orow = oh0 - 14 * c
            return rbase + cbase, orow * WO + oc, nr, ncol

        def conv1_chunks(t, s, xq, xp, xc):
            # all xq-only streams first (both chunks), then the xp/xc-
            # dependent pair streams: gives the on-chip shifted copies
            # ~4.5us to land without stalling the PE
            xb2 = xq[:, 0]
            xb1 = xq[0:CIN, 0]
            xpb = xp[:, 0]
            xcb = xc[:, 0]
            ps_c0 = psum.tile([COUT, CHUNK], dt.float32, tag="mm")
            ps_c1 = psum.tile([COUT, CHUNK], dt.float32, tag="mm")
            pst = [ps_c0, ps_c1]
            for c in range(2):
                ps1 = pst[c]
                # 9 hi streams: Whi_k*(xhi+xlo); center tap first (starts PSUM)
                for k in (4, 0, 1, 2, 3, 5, 6, 7, 8):
                    kh, kw = divmod(k, 3)
                    ro, oo, nr, ncol = tap_geom(kh, kw, c)
                    rhs = _ap(xb2, ro, [[2 * W, nr], [2, ncol]])
                    outap = (ps1[:, :] if (nr == 14 and ncol == WO)
                             else _ap(ps1[:, 0], oo, [[WO, nr], [1, ncol]]))
                    nc.tensor.matmul(outap, w1h[:, k, :], rhs,
                                     start=(k == 4), stop=False,
                                     skip_group_check=True)
                # K=64 lo: tap (0,2), plus the ow=0 sliver of tap (0,1) that
                # the column-pair stream below can't reach
                ro, oo, nr, ncol = tap_geom(0, 2, c)
                rhs = _ap(xb1, ro, [[2 * W, nr], [2, ncol]])
                nc.tensor.matmul(_ap(ps1[:, 0], oo, [[WO, nr], [1, ncol]]),
                                 w1z[:, 2, :], rhs,
                                 start=False, stop=False,
                                 skip_group_check=True)
                ro, oo, nr, _ = tap_geom(0, 1, c)
                rhs = _ap(xb1, ro, [[2 * W, nr], [2, 1]])
                nc.tensor.matmul(_ap(ps1[:, 0], oo, [[WO, nr], [1, 1]]),
                                 w1z[:, 1, :], rhs,
                                 start=False, stop=False,
                                 skip_group_check=True)
            for c in range(2):
                ps1 = pst[c]
                # 3 row-paired lo: Wlo(1,kw)*xhi[2oh] + Wlo(2,kw)*xhi[2oh+1]
                for kw in range(3):
                    if kw == 0:
                        cb, ncol, oc = 1, WO - 1, 1
                    else:
                        cb, ncol, oc = kw - 1, WO, 0
                    rhs = _ap(xpb, 28 * c * W + cb, [[2 * W, 14], [2, ncol]])
                    outap = (ps1[:, :] if ncol == WO
                             else _ap(ps1[:, 0], oc, [[WO, 14], [1, ncol]]))
                    nc.tensor.matmul(outap, w1p[:, kw, :], rhs,
                                     start=False, stop=False,
                                     skip_group_check=True)
                # 1 col-paired lo: Wlo(0,0)*xhi[2ow-1] + Wlo(0,1)*xhi[2ow],
                # rows oh>=1, cols ow>=1 (both taps valid there)
                ro, oo, nr, ncol = tap_geom(0, 0, c)
                rhs = _ap(xcb, ro, [[2 * W, nr], [2, ncol]])
                nc.tensor.matmul(_ap(ps1[:, 0], oo, [[WO, nr], [1, ncol]]),
                                 w1c[:, :], rhs,
                                 start=False, stop=True,
                                 skip_group_check=True)
                off = (t * BPC + s) * NPIX + c * CHUNK
                idx = 4 * t + 2 * s + c
                nc.scalar.copy(cv1f[:, off:off + CHUNK], ps1[:, :])
                nc.vector.bn_stats(out=st1[:, idx, :], in_=ps1[:, :])

        def sc_chunks(t, s, xb2):
            for c in range(2):
                # shortcut 1x1 stride2: rows 2oh, cols 2ow (full range)
                ps2 = psum.tile([COUT, CHUNK], dt.float32, tag="mm")
                rhs = _ap(xb2, 28 * c * W, [[2 * W, 14], [2, WO]])
                nc.tensor.matmul(ps2[:, :], wsh[:, :], rhs,
                                 start=True, stop=False, skip_group_check=True)
                nc.tensor.matmul(ps2[:, :], wsl[:, :], rhs,
                                 start=False, stop=True, skip_group_check=True)
                off = (t * BPC + s) * NPIX + c * CHUNK
                idx = 4 * t + 2 * s + c
                nc.scalar.copy(scf[:, off:off + CHUNK], ps2[:, :])
                nc.vector.bn_stats(out=sts[:, idx, :], in_=ps2[:, :])

        cc1ai = dramp.tile([COUT, 4], dt.float32)
        cc1ao = dramp.tile([COUT, 4], dt.float32, addr_space="Shared")
        cc1bi = dramp.tile([COUT, 2], dt.float32)
        cc1bo = dramp.tile([COUT, 2], dt.float32, addr_space="Shared")
        cc1ci = dramp.tile([COUT, 2], dt.float32)
        cc1co = dramp.tile([COUT, 2], dt.float32, addr_space="Shared")

        # xq needs 4 bufs: the t=6,7 tiles stay live for the deferred shortcut
        with tc.tile_pool(name="xq", bufs=4) as xpool, \
             tc.tile_pool(name="xp", bufs=2) as xppool, \
             tc.tile_pool(name="xc", bufs=2) as xcpool:
            xq_saved = []
            for t in range(T):
                for s in range(BPC):
                    xq = xpool.tile([2 * CIN, HW], dt.bfloat16, tag="xq")
                    nc.sync.dma_start(
                        out=_ap(xq[0:CIN, 0], 0, [[1, HW]]),
                        in_=xhi_d.ap()[s, :, t, :, :].rearrange("c h w -> c (h w)"))
                    nc.sync.dma_start(
                        out=_ap(xq[CIN:2 * CIN, 0], 0, [[1, HW]]),
                        in_=xlo_d.ap()[s, :, t, :, :].rearrange("c h w -> c (h w)"))
                    if t == 0 and s == 0:
                        # shortcut weights ride after the first x tile
                        nc.sync.dma_start(out=wsh[:, :], in_=wsh_d[:, :])
                        nc.sync.dma_start(out=wsl[:, :], in_=wsl_d[:, :])
                    # shifted-pair tiles [xhi; xhi(row+1)] / [xhi; xhi(col+1)]
                    # built on-chip (scalar takes the plain halves, vector
                    # the shifted ones)
                    xp = xppool.tile([2 * CIN, HW], dt.bfloat16, tag="xp")
                    nc.scalar.copy(xp[0:CIN, :], xq[0:CIN, :])
                    nc.vector.tensor_scalar_mul(
                        xp[CIN:2 * CIN, 0:HW - W], xq[0:CIN, W:HW], 1.0)
                    xc = xcpool.tile([2 * CIN, HW], dt.bfloat16, tag="xc")
                    nc.scalar.copy(xc[0:CIN, :], xq[0:CIN, :])
                    nc.vector.tensor_scalar_mul(
                        xc[CIN:2 * CIN, 0:HW - 1], xq[0:CIN, 1:HW], 1.0)
                    conv1_chunks(t, s, xq, xp, xc)
                    if t < TSPLIT:
                        sc_chunks(t, s, xq[:, 0])
                    else:
                        xq_saved.append(xq[:, 0])
                if t == TSPLIT - 1:
                    # AR1a: bn1 + scn partial (sum, sumsq) over t < TSPLIT
                    mv1a = const.tile([COUT, 2], dt.float32)
                    nc.vector.bn_aggr(out=mv1a[:, :],
                                      in_=st1[:, 0:4 * TSPLIT, :])
                    mvsa = const.tile([COUT, 2], dt.float32)
                    nc.vector.bn_aggr(out=mvsa[:, :],
                                      in_=sts[:, 0:4 * TSPLIT, :])
                    ar1a = const.tile([COUT, 4], dt.float32)
                    sums_into(ar1a[:, 0:2], mv1a, CNT_A)
                    sums_into(ar1a[:, 2:4], mvsa, CNT_A)
                    nc.sync.dma_start(out=cc1ai[:, :], in_=ar1a[:, :])
                    nc.gpsimd.collective_compute(
                        "AllReduce", Alu.add,
                        replica_groups=[list(range(NCORES))],
                        ins=[cc1ai[:, :].opt()], outs=[cc1ao[:, :].opt()])

            # AR1b: bn1 remainder (t >= TSPLIT) - phase B blocks on this one
            mv1b = const.tile([COUT, 2], dt.float32)
            nc.vector.bn_aggr(out=mv1b[:, :], in_=st1[:, 4 * TSPLIT:4 * T, :])
            ar1b = const.tile([COUT, 2], dt.float32)
            sums_into(ar1b, mv1b, CNT_B)
            nc.sync.dma_start(out=cc1bi[:, :], in_=ar1b[:, :])
            nc.gpsimd.collective_compute(
                "AllReduce", Alu.add, replica_groups=[list(range(NCORES))],
                ins=[cc1bi[:, :].opt()], outs=[cc1bo[:, :].opt()])

            # deferred shortcut for t >= TSPLIT (runs inside AR1b latency)
            for i, xb2 in enumerate(xq_saved):
                t = TSPLIT + i // BPC
                s = i % BPC
                sc_chunks(t, s, xb2)

        # AR1c: scn remainder (needed only for the mid-phase-B scf pass)
        mvsb = const.tile([COUT, 2], dt.float32)
        nc.vector.bn_aggr(out=mvsb[:, :], in_=sts[:, 4 * TSPLIT:4 * T, :])
        ar1c = const.tile([COUT, 2], dt.float32)
        sums_into(ar1c, mvsb, CNT_B)
        nc.sync.dma_start(out=cc1ci[:, :], in_=ar1c[:, :])
        nc.gpsimd.collective_compute(
            "AllReduce", Alu.add, replica_groups=[list(range(NCORES))],
            ins=[cc1ci[:, :].opt()], outs=[cc1co[:, :].opt()])

        # deferred DMAs: w2/pars needed only from phase B on
        w2 = const.tile([COUT, 9, COUT], dt.float16)
        nc.sync.dma_start(out=w2[:, :, :], in_=w2t_d[:, :, :])
        pars = const.tile([COUT, 6], dt.float32)
        nc.sync.dma_start(out=pars[:, :], in_=pars_d[:, :])

        # pars-only precomputes (vector is idle while AR1b is in flight)
        rg1 = const.tile([COUT, 1], dt.float32)    # 1/g1
        nc.vector.reciprocal(rg1[:, :], pars[:, 0:1])
        rgb = const.tile([COUT, 1], dt.float32)    # b1/g1
        nc.vector.tensor_tensor(rgb[:, :], pars[:, 1:2], rg1[:, :], Alu.mult)
        rg1h = const.tile([COUT, 1], dt.float32)   # 0.5/g1
        nc.vector.tensor_scalar_mul(rg1h[:, :], rg1[:, :], 0.5)
        rg2 = const.tile([COUT, 1], dt.float32)    # (0.5-b1)/g1
        nc.vector.tensor_tensor(rg2[:, :], rg1h[:, :], rgb[:, :], Alu.subtract)
        halfmb = const.tile([COUT, 1], dt.float32)  # 0.5 - scn_b
        nc.vector.tensor_scalar(halfmb[:, :], pars[:, 5:6], -1.0, 0.5,
                                Alu.mult, Alu.add)

        def bn_sums(tag, parts, fn=Act.Sqrt):
            """Combine AllReduced (mean, msq) partials -> mean, fn(var+eps)."""
            tot = const.tile([COUT, 2], dt.float32, tag=tag + "_tot")
            nc.vector.tensor_tensor(tot[:, :], parts[0], parts[1], Alu.add)
            if len(parts) > 2:
                nc.vector.tensor_tensor(tot[:, :], tot[:, :], parts[2], Alu.add)
            mean = tot[:, 0:1]
            var = const.tile([COUT, 1], dt.float32, tag=tag + "_var")
            nc.vector.tensor_tensor(var[:, :], mean, mean, Alu.mult)
            nc.vector.tensor_tensor(var[:, :], tot[:, 1:2], var[:, :],
                                    Alu.subtract)
            std = const.tile([COUT, 1], dt.float32, tag=tag + "_std")
            nc.scalar.activation(std[:, :], var[:, :], fn, bias=eps_t[:, :])
            return mean, std

        # ---- bn1 consts (vector stalls here on AR1b only) ----
        gs1a = const.tile([COUT, 4], dt.float32)
        nc.sync.dma_start(out=gs1a[:, :], in_=cc1ao[:, :])
        gs1b = const.tile([COUT, 2], dt.float32)
        nc.sync.dma_start(out=gs1b[:, :], in_=cc1bo[:, :])
        gs1c = const.tile([COUT, 2], dt.float32)
        nc.sync.dma_start(out=gs1c[:, :], in_=cc1co[:, :])

        m1, std1 = bn_sums("bn1", (gs1a[:, 0:2], gs1b[:, :]))
        # tau = 0.5*std1/g1 ; btil = b1*std1/g1 - mean1 ;
        # tau2 = tau - btil = (0.5-b1)*std1/g1 + mean1 (raw-space threshold,
        # computed first so spike(0) fires as early as possible)
        tau2 = const.tile([COUT, 1], dt.float32)
        nc.vector.scalar_tensor_tensor(tau2[:, :], std1[:, :], rg2[:, :],
                                       m1[:, :], Alu.mult, Alu.add)
        tau = const.tile([COUT, 1], dt.float32)
        nc.vector.tensor_tensor(tau[:, :], std1[:, :], rg1h[:, :], Alu.mult)
        btil = const.tile([COUT, 1], dt.float32)
        nc.vector.scalar_tensor_tensor(btil[:, :], std1[:, :], rgb[:, :],
                                       m1[:, :], Alu.mult, Alu.subtract)

        # ============ phase B: LIF recurrence + conv2 (fp16, partial) =======
        def fold(t):  # c' = cv1 + btil, in place, one fused (s-pair) slice
            sl = cv1f[:, t * SPT:(t + 1) * SPT]
            nc.scalar.activation(sl, sl, Act.Identity, bias=btil[:, :])

        cc2ai = dramp.tile([COUT, 2], dt.float32)
        cc2ao = dramp.tile([COUT, 2], dt.float32, addr_space="Shared")
        cc2bi = dramp.tile([COUT, 4], dt.float32)
        cc2bo = dramp.tile([COUT, 4], dt.float32, addr_space="Shared")
        ar2b = const.tile([COUT, 4], dt.float32, tag="ar2b")

        with tc.tile_pool(name="pu", bufs=2) as pu, \
             tc.tile_pool(name="pv", bufs=2) as pv, \
             tc.tile_pool(name="psp", bufs=3) as psp:

            def spike(v_ap, thr):  # contiguous fp16 {0,1} tile, both samples
                sq = psp.tile([COUT, SPT], dt.float16, tag="sq")
                nc.vector.tensor_scalar(sq[:, :], v_ap, thr[:, :], None,
                                        Alu.is_gt)
                return sq

            # spike(0) straight off raw cv1 (tau2), before fold(0) lands
            sq = spike(cv1f[:, 0:SPT], tau2)
            fold(0)
            v_prev = cv1f[:, 0:SPT]
            for t in range(T):
                if t + 1 < T:
                    fold(t + 1)
                    u = pu.tile([COUT, SPT], dt.float32, tag="u")
                    nc.vector.scalar_tensor_tensor(
                        u[:, :], v_prev, tau[:, :], v_prev, Alu.is_le, Alu.mult)
                    v = pv.tile([COUT, SPT], dt.float32, tag="v")
                    nc.vector.scalar_tensor_tensor(
                        v[:, :], u[:, :], float(d),
                        cv1f[:, (t + 1) * SPT:(t + 2) * SPT], Alu.mult, Alu.add)
                    v_prev = v[:, :]
                    sq_next = spike(v_prev, tau)
                else:
                    sq_next = None
                sqb = sq[:, 0]
                for s in range(BPC):
                    for c in range(2):
                        ps3 = psum.tile([COUT, CHUNK], dt.float32, tag="mm")
                        so = s * NPIX
                        oh0 = 14 * c
                        for ki, k in enumerate((4, 0, 1, 2, 3, 5, 6, 7, 8)):
                            kh, kw = divmod(k, 3)
                            r0 = oh0 + kh - 1
                            nr, o_r = 14, 0
                            if r0 < 0:          # kh=0, c=0
                                r0, nr, o_r = 0, 13, 1
                            elif r0 + 13 > 27:  # kh=2, c=1
                                nr = 13
                            if kw == 0:
                                cb, ncol, o_c = 0, WO - 1, 1
                            elif kw == 2:
                                cb, ncol, o_c = 1, WO - 1, 0
                            else:
                                cb, ncol, o_c = 0, WO, 0
                            outap = (ps3[:, :] if (nr == 14 and ncol == WO)
                                     else _ap(ps3[:, 0], o_r * WO + o_c,
                                              [[WO, nr], [1, ncol]]))
                            nc.tensor.matmul(
                                outap, w2[:, k, :],
                                _ap(sqb, so + r0 * WO + cb,
                                    [[WO, nr], [1, ncol]]),
                                start=(ki == 0), stop=(ki == 8),
                                skip_group_check=True)
                        off = (t * BPC + s) * NPIX + c * CHUNK
                        idx = 4 * t + 2 * s + c
                        nc.scalar.copy(cv2f[:, off:off + CHUNK], ps3[:, :])
                        nc.vector.bn_stats(out=st2[:, idx, :], in_=ps3[:, :])
                if t == 2:
                    # scn consts + scf threshold pass on the idle gpsimd
                    # (AR1c has landed by now; queues reach here late enough
                    # not to stall the LIF chain)
                    msc, stdsc = bn_sums("scn", (gs1a[:, 2:4], gs1c[:, :]))
                    rstds = const.tile([COUT, 1], dt.float32)
                    nc.vector.reciprocal(rstds[:, :], stdsc[:, :])
                    asc = const.tile([COUT, 1], dt.float32)
                    nc.vector.tensor_tensor(asc[:, :], pars[:, 4:5],
                                            rstds[:, :], Alu.mult)
                    nasc = const.tile([COUT, 1], dt.float32)
                    nc.vector.tensor_scalar_mul(nasc[:, :], asc[:, :], -1.0)
                    c1t = const.tile([COUT, 1], dt.float32)
                    nc.vector.scalar_tensor_tensor(
                        c1t[:, :], asc[:, :], msc[:, :], halfmb[:, :],
                        Alu.mult, Alu.add)
                if 3 <= t < 7:
                    # scf <- -asc*scf + c1t on SCALAR (gpsimd's version of
                    # this pass starves concurrent DVE SBUF access)
                    q0 = (t - 3) * (NLOC // 4)
                    sl = scf[:, q0:q0 + NLOC // 4]
                    nc.scalar.activation(sl, sl, Act.Identity,
                                         bias=c1t[:, :], scale=nasc[:, :])
                if t == A2SPLIT - 1:
                    # AR2a: bn2 partial, hidden under conv2 t=5,6,7
                    mv2a = const.tile([COUT, 2], dt.float32)
                    nc.vector.bn_aggr(out=mv2a[:, :],
                                      in_=st2[:, 0:4 * A2SPLIT, :])
                    ar2a = const.tile([COUT, 2], dt.float32)
                    sums_into(ar2a, mv2a, CNT_2A)
                    nc.sync.dma_start(out=cc2ai[:, :], in_=ar2a[:, :])
                    nc.gpsimd.collective_compute(
                        "AllReduce", Alu.add,
                        replica_groups=[list(range(NCORES))],
                        ins=[cc2ai[:, :].opt()], outs=[cc2ao[:, :].opt()])
                if t == T - 2:
                    # pre-aggregate t=5,6 for AR2b during t=7's compute, so
                    # only t=7's 4 chunks sit on the final trigger chain
                    mv2b1 = const.tile([COUT, 2], dt.float32)
                    nc.vector.bn_aggr(out=mv2b1[:, :],
                                      in_=st2[:, 4 * A2SPLIT:4 * (T - 1), :])
                    sums_into(ar2b[:, 0:2], mv2b1, 4 * (T - 1 - A2SPLIT) * CHUNK)
                sq = sq_next

        # ---- AR2b (bn2 remainder) - the only exposed tail collective ----
        mv2b = const.tile([COUT, 2], dt.float32)
        nc.vector.bn_aggr(out=mv2b[:, :], in_=st2[:, 4 * (T - 1):4 * T, :])
        sums_into(ar2b[:, 2:4], mv2b, 4 * CHUNK)
        nc.sync.dma_start(out=cc2bi[:, :], in_=ar2b[:, :])
        nc.gpsimd.collective_compute(
            "AllReduce", Alu.add, replica_groups=[list(range(NCORES))],
            ins=[cc2bi[:, :].opt()], outs=[cc2bo[:, :].opt()])

        gs2a = const.tile([COUT, 2], dt.float32)
        nc.sync.dma_start(out=gs2a[:, :], in_=cc2ao[:, :])
        gs2b = const.tile([COUT, 4], dt.float32)
        nc.sync.dma_start(out=gs2b[:, :], in_=cc2bo[:, :])

        m2v, std2 = bn_sums("bn2", (gs2a[:, :], gs2b[:, 0:2], gs2b[:, 2:4]))
        a2 = const.tile([COUT, 1], dt.float32)
        nc.vector.reciprocal(a2[:, :], std2[:, :])
        nc.vector.tensor_tensor(a2[:, :], a2[:, :], pars[:, 2:3], Alu.mult)
        b2 = const.tile([COUT, 1], dt.float32)
        nc.vector.tensor_tensor(b2[:, :], a2[:, :], m2v[:, :], Alu.mult)
        nc.vector.tensor_tensor(b2[:, :], pars[:, 3:4], b2[:, :], Alu.subtract)

        # epilogue: z' = a2*cv2 + b2 on scalar (z-rate 1.68us < cmp 1.78us,
        # so the vector cmp chain is the limiter either way; gpsimd versions
        # of either pass starve DVE SBUF access); compares on vector (the
        # only engine allowing fp32-in uint8-out cmp)
        with tc.tile_pool(name="outp", bufs=3) as op, \
             tc.tile_pool(name="zp", bufs=3) as zp:
            for t in range(T):
                off = t * SPT
                z = zp.tile([COUT, SPT], dt.float32, tag="z")
                nc.scalar.activation(z[:, :], cv2f[:, off:off + SPT],
                                     Act.Identity, bias=b2[:, :],
                                     scale=a2[:, :])
                ot = op.tile([COUT, SPT], dt.uint8, tag="ot")
                nc.vector.tensor_tensor(ot[:, :], z[:, :],
                                        scf[:, off:off + SPT], Alu.is_gt)
                for s in range(BPC):
                    nc.sync.dma_start(
                        out=out_d.ap()[s, :, t, :, :].rearrange("c h w -> c (h w)"),
                        in_=ot[:, s * NPIX:(s + 1) * NPIX])

    nc.compile()
    return nc


_CACHE = {}


def _bf16_hilo(a):
    import ml_dtypes
    a = np.asarray(a, np.float32)
    hi = a.astype(ml_dtypes.bfloat16)
    lo = (a - hi.astype(np.float32)).astype(ml_dtypes.bfloat16)
    return hi, lo


def _host_prep(inputs):
    xhi, xlo = _bf16_hilo(inputs["x"])
    xhi, xlo = np.ascontiguousarray(xhi), np.ascontiguousarray(xlo)
    w1t = np.ascontiguousarray(inputs["cv1_w"], np.float32).reshape(
        COUT, CIN, 3, 3).transpose(1, 2, 3, 0).reshape(CIN, 9, COUT)
    w1hi, w1lo = _bf16_hilo(w1t)
    w1h = np.ascontiguousarray(np.concatenate([w1hi, w1hi], axis=0))
    # paired lo weights: [Wlo(1,kw); Wlo(2,kw)] stacked on K
    w1p = np.ascontiguousarray(np.concatenate(
        [w1lo[:, 3:6, :], w1lo[:, 6:9, :]], axis=0))
    w1z = np.ascontiguousarray(w1lo[:, 0:3, :])
    w1c = np.ascontiguousarray(np.concatenate(
        [w1lo[:, 0, :], w1lo[:, 1, :]], axis=0))
    wst = np.asarray(inputs["sc_w"], np.float32).reshape(COUT, CIN).T
    wshi, wslo = _bf16_hilo(wst)
    wsh = np.ascontiguousarray(np.concatenate([wshi, wshi], axis=0))
    wsl = np.ascontiguousarray(np.concatenate([wslo, wslo], axis=0))
    w2t = np.ascontiguousarray(inputs["cv2_w"], np.float32).reshape(
        COUT, COUT, 3, 3).transpose(1, 2, 3, 0).reshape(COUT, 9, COUT)
    w2t = np.ascontiguousarray(w2t.astype(np.float16))
    pars = np.ascontiguousarray(np.stack(
        [np.asarray(inputs[p], np.float32).ravel()
         for p in ["bn1_g", "bn1_b", "bn2_g", "bn2_b", "scn_g", "scn_b"]],
        axis=1))
    d = float(1.0 / (1.0 + math.exp(-float(np.asarray(inputs["decay"]).ravel()[0]))))

    in_maps = []
    for c in range(NCORES):
        m = {"xhi": xhi[c * BPC:(c + 1) * BPC], "xlo": xlo[c * BPC:(c + 1) * BPC],
             "w1h": w1h, "w1p": w1p, "w1z": w1z, "w1c": w1c,
             "wsh": wsh, "wsl": wsl, "w2t": w2t, "pars": pars}
        in_maps.append(m)
    return in_maps, d


def kernel(**inputs):
    in_maps, d = _host_prep(inputs)
    key = round(d, 12)
    if key not in _CACHE:
        _CACHE[key] = build_nc(d)
    nc = _CACHE[key]

    res = run_bass_kernel_spmd(nc, in_maps, core_ids=list(range(NCORES)))
    out = np.concatenate([res.results[c]["out"] for c in range(NCORES)], axis=0)
    return np.ascontiguousarray(out, dtype=np.float32)


# revision 37
# speedup vs baseline: 1.0137x; 1.0137x over previous
"""LIAFResBlock forward on 8 Trainium2 NeuronCores (data-parallel over batch).

Self-contained: hardcodes shapes for x [16,64,8,56,56] -> out [16,128,8,28,28].

Math notes (vs the PyTorch/JAX reference):
  - conv biases are no-ops: every conv is followed by training-mode BN, which
    subtracts the per-channel mean, absorbing any per-channel constant.
  - the final mem_update on a binary {0,1} tensor is the identity because
    d = sigmoid(0.5) ~ 0.6225 and d*0.5 < 0.5, so out = lif_act(bn2(cv2)+bn_sc(sc)).
  - the first mem_update runs in "normalized" space: with a1 = g1*rstd1 (>0),
    v = m/a1 satisfies v[t] = d*v[t-1]*[v<=tau] + (cv1[t] + beta1/a1),
    spike[t] = v[t] > tau, tau = 0.5/a1. BN1 folds into a per-channel bias on
    cv1 plus a per-channel threshold.
  - BN batch stats are global over B=16: each core computes per-channel
    (sum, sumsq) partials; tiny AllReduces combine them.
  - shortcut threshold: scf <- -asc*scf + (0.5-bsc) on gpsimd during phase B;
    final out = 1[a2*cv2 + b2 > scf] elementwise.

Conv1 structure (phase A, per 392-px chunk = 15 streams instead of 18):
  x = xhi + xlo and W1 = Whi + Wlo (bf16 splits; products exact in fp32 PSUM).
  - 9 streams [Whi_k;Whi_k] (x) [xhi;xlo]    -> Whi_k*(xhi+xlo), all taps
  - 3 streams [Wlo(1,kw);Wlo(2,kw)] (x) [xhi;xhi_rowshift] -> row-adjacent
    tap pair of the lo-correction in ONE K=128 stream (the shifted copy is
    built on-chip: scalar copies the hi half, vector the shifted half)
  - 3 streams Wlo(0,kw) (x) xhi, K=64 (partial rows, oh>=1)
  The Wlo*xlo term (~2^-18 relative) is dropped.

Schedule notes (AllReduce latency ~8-12us + core launch skew ~17us):
  - AR1a: bn1+scn partial sums over t=0..5, hidden under conv1 t=6,7.
  - AR1b: bn1 remainder, issued the moment conv1 finishes; deferred t=6,7
    shortcut matmuls run inside its latency. Phase B blocks only on AR1b.
  - AR1c: scn remainder, lands early in phase B; scf threshold pass runs on
    the otherwise-idle gpsimd mid phase B.
  - AR2a: bn2 partial (t=0..5) hidden under conv2 t=6,7; AR2b (t=6,7) at
    phase B end is the only exposed tail latency.
  - epilogue: z' = a2*cv2+b2 on scalar (t<7) / gpsimd (t=7), compare on
    vector (t<7) / gpsimd (t=7), fp16 {0,1} output (host converts).

Implementation notes (hardware-measured):
  - conv2 fp16 single-pass (spikes are exact {0,1} in fp16; w2 fp16 rounding
    costs ~590 flips).
  - zero padding via partial-range matmuls (each off-center tap writes only
    its valid output sub-rectangle of PSUM), NOT zero-ringed input tiles
    (those force 224-byte DMA descriptors).
  - spikes are written as contiguous [128, 1568] fp16 on Vector (gpsimd's
    fp16 store path is ~16ns/elem and starves concurrent DVE SBUF access).
  - weights are transposed host-side so every DMA is contiguous.
"""
import math
import sys

import numpy as np

sys.path.insert(0, "/opt/trn_rl_repo")

import concourse.bass as bass  # noqa: E402
import concourse.bacc as bacc  # noqa: E402
import concourse.tile as tile  # noqa: E402
from concourse import mybir  # noqa: E402
from concourse.bass_utils import run_bass_kernel_spmd  # noqa: E402

dt = mybir.dt
Alu = mybir.AluOpType
Act = mybir.ActivationFunctionType

B, CIN, COUT, T, H, W = 16, 64, 128, 8, 56, 56
HO = WO = 28
NPIX = HO * WO          # 784
CHUNK = NPIX // 2       # 392 (one PSUM bank)
NCORES = 8
BPC = B // NCORES       # 2 samples per core
SPT = BPC * NPIX        # 1568 elements per fused (both-samples) timestep
NLOC = BPC * T * NPIX   # 12544 elements/channel per core
NGLOB = B * T * NPIX    # 100352 elements/channel globally
EPS = 1e-5
HW = H * W              # 3136 elements per unpadded input plane
TSPLIT = 6              # AR1 split: t < TSPLIT early, rest late
CNT_A = 4 * TSPLIT * CHUNK        # elems/channel/core in the early partial
CNT_B = 4 * (T - TSPLIT) * CHUNK  # and in the late partial
A2SPLIT = 5             # AR2 split: earlier, so AR2a clears the serial cc
CNT_2A = 4 * A2SPLIT * CHUNK      # stream well before AR2b triggers
CNT_2B = 4 * (T - A2SPLIT) * CHUNK


def _ap(base, off, free):
    """Sub-view of an SBUF AP: keep partition dim, custom free dims."""
    return bass.AP(tensor=base.tensor, offset=base.offset + off,
                   ap=[base.ap[0]] + free)


def build_nc(d: float) -> bass.Bass:
    nc = bacc.Bacc("TRN2", target_bir_lowering=False, num_devices=NCORES)

    xhi_d = nc.dram_tensor("xhi", [BPC, CIN, T, H, W], dt.bfloat16,
                           kind="ExternalInput")
    xlo_d = nc.dram_tensor("xlo", [BPC, CIN, T, H, W], dt.bfloat16,
                           kind="ExternalInput")
    # host-pretransposed lhsT weights
    w1h_d = nc.dram_tensor("w1h", [2 * CIN, 9, COUT], dt.bfloat16,
                           kind="ExternalInput")   # [Whi; Whi] per tap
    w1p_d = nc.dram_tensor("w1p", [2 * CIN, 3, COUT], dt.bfloat16,
                           kind="ExternalInput")   # [Wlo(1,kw); Wlo(2,kw)]
    w1z_d = nc.dram_tensor("w1z", [CIN, 3, COUT], dt.bfloat16,
                           kind="ExternalInput")   # Wlo(0,kw)
    w1c_d = nc.dram_tensor("w1c", [2 * CIN, COUT], dt.bfloat16,
                           kind="ExternalInput")   # [Wlo(0,0); Wlo(0,1)]
    wsh_d = nc.dram_tensor("wsh", [2 * CIN, COUT], dt.bfloat16,
                           kind="ExternalInput")
    wsl_d = nc.dram_tensor("wsl", [2 * CIN, COUT], dt.bfloat16,
                           kind="ExternalInput")
    w2t_d = nc.dram_tensor("w2t", [COUT, 9, COUT], dt.float16,
                           kind="ExternalInput")
    # columns: bn1_g, bn1_b, bn2_g, bn2_b, scn_g, scn_b
    pars_d = nc.dram_tensor("pars", [COUT, 6], dt.float32, kind="ExternalInput")
    out_d = nc.dram_tensor("out", [BPC, COUT, T, HO, WO], dt.uint8,
                           kind="ExternalOutput")

    from contextlib import ExitStack
    with tile.TileContext(nc) as tc, ExitStack() as stk:
        big = stk.enter_context(tc.tile_pool(name="big", bufs=1))
        const = stk.enter_context(tc.tile_pool(name="const", bufs=1))
        psum = stk.enter_context(tc.tile_pool(name="psum", bufs=8, space="PSUM"))
        dramp = stk.enter_context(tc.tile_pool(name="dramp", bufs=1, space="DRAM"))

        # warmup AllReduce first: starts the one-time cc-stream init (~60us)
        # as early as possible, fully overlapped with phase A
        wrm = const.tile([1, 1], dt.float32)
        nc.vector.memset(wrm[:, :], 0.0)
        ccwi = dramp.tile([1, 1], dt.float32)
        ccwo = dramp.tile([1, 1], dt.float32, addr_space="Shared")
        nc.sync.dma_start(out=ccwi[:, :], in_=wrm[:, :])
        nc.gpsimd.collective_compute(
            "AllReduce", Alu.add, replica_groups=[list(range(NCORES))],
            ins=[ccwi[:, :].opt()], outs=[ccwo[:, :].opt()])

        # ---- conv1 weights first (first matmul needs only w1h + x tile) ----
        w1h = const.tile([2 * CIN, 9, COUT], dt.bfloat16)
        nc.sync.dma_start(out=w1h[:, :, :], in_=w1h_d[:, :, :])
        w1p = const.tile([2 * CIN, 3, COUT], dt.bfloat16)
        nc.sync.dma_start(out=w1p[:, :, :], in_=w1p_d[:, :, :])
        w1z = const.tile([CIN, 3, COUT], dt.bfloat16)
        nc.sync.dma_start(out=w1z[:, :, :], in_=w1z_d[:, :, :])
        w1c = const.tile([2 * CIN, COUT], dt.bfloat16)
        nc.sync.dma_start(out=w1c[:, :], in_=w1c_d[:, :])
        wsh = const.tile([2 * CIN, COUT], dt.bfloat16)
        wsl = const.tile([2 * CIN, COUT], dt.bfloat16)
        eps_t = const.tile([COUT, 1], dt.float32)
        nc.vector.memset(eps_t[:, :], EPS)

        # ---- persistent activation buffers, (t, s)-major free layout ----
        cv1f = big.tile([COUT, NLOC], dt.float32)   # conv1 raw, then c'=cv1+btil
        scf = big.tile([COUT, NLOC], dt.float32)    # shortcut raw, then thr
        cv2f = big.tile([COUT, NLOC], dt.float32)   # conv2 raw
        st1 = const.tile([COUT, 4 * T, 6], dt.float32)
        sts = const.tile([COUT, 4 * T, 6], dt.float32)
        st2 = const.tile([COUT, 4 * T, 6], dt.float32)

        def sums_into(dst2, mv, cnt):
            """(mean,var)[128,2] -> (sum/NGLOB, sumsq/NGLOB) into dst2.

            Pre-scaled by 1/NGLOB so the AllReduced totals are directly the
            global mean and mean-square."""
            w = float(cnt) / NGLOB
            nc.vector.tensor_scalar_mul(dst2[:, 0:1], mv[:, 0:1], w)
            nc.vector.scalar_tensor_tensor(
                dst2[:, 1:2], mv[:, 0:1], w, mv[:, 0:1],
                Alu.mult, Alu.mult)
            nc.vector.scalar_tensor_tensor(
                dst2[:, 1:2], mv[:, 1:2], w, dst2[:, 1:2],
                Alu.mult, Alu.add)

        # ===== phase A: conv1 + shortcut (bf16 hi/lo, partial-range) ========
        # conv1 tap (kh,kw): input (2oh+kh-1, 2ow+kw-1); kh=0 needs oh>=1,
        # kw=0 needs ow>=1, all else full. Chunk c covers oh in [14c, 14c+14).
        def tap_geom(kh, kw, c):
            if kh == 0:
                oh0 = 1 if c == 0 else 14
                nr = 13 if c == 0 else 14
            else:
                oh0, nr = 14 * c, 14
            rbase = (2 * oh0 + kh - 1) * W
            if kw == 0:
                cbase, ncol, oc = 1, WO - 1, 1
            else:
                cbase, ncol, oc = kw - 1, WO, 0
            orow = oh0 - 14 * c
            return rbase + cbase, orow * WO + oc, nr, ncol

        def conv1_chunks(t, s, xq, xp, xc):
            # all xq-only streams first (both chunks), then the xp/xc-
            # dependent pair streams: gives the on-chip shifted copies
            # ~4.5us to land without stalling the PE
            xb2 = xq[:, 0]
            xb1 = xq[0:CIN, 0]
            xpb = xp[:, 0]
            xcb = xc[:, 0]
            ps_c0 = psum.tile([COUT, CHUNK], dt.float32, tag="mm")
            ps_c1 = psum.tile([COUT, CHUNK], dt.float32, tag="mm")
            pst = [ps_c0, ps_c1]
            for c in range(2):
                ps1 = pst[c]
                # 9 hi streams: Whi_k*(xhi+xlo); center tap first (starts PSUM)
                for k in (4, 0, 1, 2, 3, 5, 6, 7, 8):
                    kh, kw = divmod(k, 3)
                    ro, oo, nr, ncol = tap_geom(kh, kw, c)
                    rhs = _ap(xb2, ro, [[2 * W, nr], [2, ncol]])
                    outap = (ps1[:, :] if (nr == 14 and ncol == WO)
                             else _ap(ps1[:, 0], oo, [[WO, nr], [1, ncol]]))
                    nc.tensor.matmul(outap, w1h[:, k, :], rhs,
                                     start=(k == 4), stop=False,
                                     skip_group_check=True)
                # K=64 lo: tap (0,2), plus the ow=0 sliver of tap (0,1) that
                # the column-pair stream below can't reach
                ro, oo, nr, ncol = tap_geom(0, 2, c)
                rhs = _ap(xb1, ro, [[2 * W, nr], [2, ncol]])
                nc.tensor.matmul(_ap(ps1[:, 0], oo, [[WO, nr], [1, ncol]]),
                                 w1z[:, 2, :], rhs,
                                 start=False, stop=False,
                                 skip_group_check=True)
                ro, oo, nr, _ = tap_geom(0, 1, c)
                rhs = _ap(xb1, ro, [[2 * W, nr], [2, 1]])
                nc.tensor.matmul(_ap(ps1[:, 0], oo, [[WO, nr], [1, 1]]),
                                 w1z[:, 1, :], rhs,
                                 start=False, stop=False,
                                 skip_group_check=True)
            for c in range(2):
                ps1 = pst[c]
                # 3 row-paired lo: Wlo(1,kw)*xhi[2oh] + Wlo(2,kw)*xhi[2oh+1]
                for kw in range(3):
                    if kw == 0:
                        cb, ncol, oc = 1, WO - 1, 1
                    else:
                        cb, ncol, oc = kw - 1, WO, 0
                    rhs = _ap(xpb, 28 * c * W + cb, [[2 * W, 14], [2, ncol]])
                    outap = (ps1[:, :] if ncol == WO
                             else _ap(ps1[:, 0], oc, [[WO, 14], [1, ncol]]))
                    nc.tensor.matmul(outap, w1p[:, kw, :], rhs,
                                     start=False, stop=False,
                                     skip_group_check=True)
                # 1 col-paired lo: Wlo(0,0)*xhi[2ow-1] + Wlo(0,1)*xhi[2ow],
                # rows oh>=1, cols ow>=1 (both taps valid there)
                ro, oo, nr, ncol = tap_geom(0, 0, c)
                rhs = _ap(xcb, ro, [[2 * W, nr], [2, ncol]])
                nc.tensor.matmul(_ap(ps1[:, 0], oo, [[WO, nr], [1, ncol]]),
                                 w1c[:, :], rhs,
                                 start=False, stop=True,
                                 skip_group_check=True)
                off = (t * BPC + s) * NPIX + c * CHUNK
                idx = 4 * t + 2 * s + c
                nc.scalar.copy(cv1f[:, off:off + CHUNK], ps1[:, :])
                nc.vector.bn_stats(out=st1[:, idx, :], in_=ps1[:, :])

        def sc_chunks(t, s, xb2):
            for c in range(2):
                # shortcut 1x1 stride2: rows 2oh, cols 2ow (full range)
                ps2 = psum.tile([COUT, CHUNK], dt.float32, tag="mm")
                rhs = _ap(xb2, 28 * c * W, [[2 * W, 14], [2, WO]])
                nc.tensor.matmul(ps2[:, :], wsh[:, :], rhs,
                                 start=True, stop=False, skip_group_check=True)
                nc.tensor.matmul(ps2[:, :], wsl[:, :], rhs,
                                 start=False, stop=True, skip_group_check=True)
                off = (t * BPC + s) * NPIX + c * CHUNK
                idx = 4 * t + 2 * s + c
                nc.scalar.copy(scf[:, off:off + CHUNK], ps2[:, :])
                nc.vector.bn_stats(out=sts[:, idx, :], in_=ps2[:, :])

        cc1ai = dramp.tile([COUT, 4], dt.float32)
        cc1ao = dramp.tile([COUT, 4], dt.float32, addr_space="Shared")
        cc1bi = dramp.tile([COUT, 2], dt.float32)
        cc1bo = dramp.tile([COUT, 2], dt.float32, addr_space="Shared")
        cc1ci = dramp.tile([COUT, 2], dt.float32)
        cc1co = dramp.tile([COUT, 2], dt.float32, addr_space="Shared")

        # xq needs 4 bufs: the t=6,7 tiles stay live for the deferred shortcut
        with tc.tile_pool(name="xq", bufs=4) as xpool, \
             tc.tile_pool(name="xp", bufs=2) as xppool, \
             tc.tile_pool(name="xc", bufs=2) as xcpool:
            xq_saved = []
            for t in range(T):
                for s in range(BPC):
                    xq = xpool.tile([2 * CIN, HW], dt.bfloat16, tag="xq")
                    nc.sync.dma_start(
                        out=_ap(xq[0:CIN, 0], 0, [[1, HW]]),
                        in_=xhi_d.ap()[s, :, t, :, :].rearrange("c h w -> c (h w)"))
                    nc.sync.dma_start(
                        out=_ap(xq[CIN:2 * CIN, 0], 0, [[1, HW]]),
                        in_=xlo_d.ap()[s, :, t, :, :].rearrange("c h w -> c (h w)"))
                    if t == 0 and s == 0:
                        # shortcut weights ride after the first x tile
                        nc.sync.dma_start(out=wsh[:, :], in_=wsh_d[:, :])
                        nc.sync.dma_start(out=wsl[:, :], in_=wsl_d[:, :])
                    # shifted-pair tiles [xhi; xhi(row+1)] / [xhi; xhi(col+1)]
                    # built on-chip; the copy work is spread over scalar,
                    # vector and a SBUF->SBUF DMA so no engine becomes the
                    # per-slice bottleneck
                    xp = xppool.tile([2 * CIN, HW], dt.bfloat16, tag="xp")
                    xc = xcpool.tile([2 * CIN, HW], dt.bfloat16, tag="xc")
                    nc.scalar.copy(xp[0:CIN, :], xq[0:CIN, :])
                    nc.sync.dma_start(out=xc[0:CIN, :], in_=xq[0:CIN, :])
                    if (t * BPC + s) % 2 == 0:
                        nc.vector.tensor_scalar_mul(
                            xp[CIN:2 * CIN, 0:HW - W], xq[0:CIN, W:HW], 1.0)
                        nc.vector.tensor_scalar_mul(
                            xc[CIN:2 * CIN, 0:HW - 1], xq[0:CIN, 1:HW], 1.0)
                    else:
                        nc.vector.tensor_scalar_mul(
                            xp[CIN:2 * CIN, 0:HW - W], xq[0:CIN, W:HW], 1.0)
                        nc.scalar.copy(xc[CIN:2 * CIN, 0:HW - 1],
                                       xq[0:CIN, 1:HW])
                    conv1_chunks(t, s, xq, xp, xc)
                    if t < TSPLIT:
                        sc_chunks(t, s, xq[:, 0])
                    else:
                        xq_saved.append(xq[:, 0])
                if t == TSPLIT - 1:
                    # AR1a: bn1 + scn partial (sum, sumsq) over t < TSPLIT
                    mv1a = const.tile([COUT, 2], dt.float32)
                    nc.vector.bn_aggr(out=mv1a[:, :],
                                      in_=st1[:, 0:4 * TSPLIT, :])
                    mvsa = const.tile([COUT, 2], dt.float32)
                    nc.vector.bn_aggr(out=mvsa[:, :],
                                      in_=sts[:, 0:4 * TSPLIT, :])
                    ar1a = const.tile([COUT, 4], dt.float32)
                    sums_into(ar1a[:, 0:2], mv1a, CNT_A)
                    sums_into(ar1a[:, 2:4], mvsa, CNT_A)
                    nc.sync.dma_start(out=cc1ai[:, :], in_=ar1a[:, :])
                    nc.gpsimd.collective_compute(
                        "AllReduce", Alu.add,
                        replica_groups=[list(range(NCORES))],
                        ins=[cc1ai[:, :].opt()], outs=[cc1ao[:, :].opt()])

            # AR1b: bn1 remainder (t >= TSPLIT) - phase B blocks on this one
            mv1b = const.tile([COUT, 2], dt.float32)
            nc.vector.bn_aggr(out=mv1b[:, :], in_=st1[:, 4 * TSPLIT:4 * T, :])
            ar1b = const.tile([COUT, 2], dt.float32)
            sums_into(ar1b, mv1b, CNT_B)
            nc.sync.dma_start(out=cc1bi[:, :], in_=ar1b[:, :])
            nc.gpsimd.collective_compute(
                "AllReduce", Alu.add, replica_groups=[list(range(NCORES))],
                ins=[cc1bi[:, :].opt()], outs=[cc1bo[:, :].opt()])

            # deferred shortcut for t >= TSPLIT (runs inside AR1b latency)
            for i, xb2 in enumerate(xq_saved):
                t = TSPLIT + i // BPC
                s = i % BPC
                sc_chunks(t, s, xb2)

        # AR1c: scn remainder (needed only for the mid-phase-B scf pass)
        mvsb = const.tile([COUT, 2], dt.float32)
        nc.vector.bn_aggr(out=mvsb[:, :], in_=sts[:, 4 * TSPLIT:4 * T, :])
        ar1c = const.tile([COUT, 2], dt.float32)
        sums_into(ar1c, mvsb, CNT_B)
        nc.sync.dma_start(out=cc1ci[:, :], in_=ar1c[:, :])
        nc.gpsimd.collective_compute(
            "AllReduce", Alu.add, replica_groups=[list(range(NCORES))],
            ins=[cc1ci[:, :].opt()], outs=[cc1co[:, :].opt()])

        # deferred DMAs: w2/pars needed only from phase B on
        w2 = const.tile([COUT, 9, COUT], dt.float16)
        nc.sync.dma_start(out=w2[:, :, :], in_=w2t_d[:, :, :])
        pars = const.tile([COUT, 6], dt.float32)
        nc.sync.dma_start(out=pars[:, :], in_=pars_d[:, :])

        # pars-only precomputes (vector is idle while AR1b is in flight)
        rg1 = const.tile([COUT, 1], dt.float32)    # 1/g1
        nc.vector.reciprocal(rg1[:, :], pars[:, 0:1])
        rgb = const.tile([COUT, 1], dt.float32)    # b1/g1
        nc.vector.tensor_tensor(rgb[:, :], pars[:, 1:2], rg1[:, :], Alu.mult)
        rg1h = const.tile([COUT, 1], dt.float32)   # 0.5/g1
        nc.vector.tensor_scalar_mul(rg1h[:, :], rg1[:, :], 0.5)
        rg2 = const.tile([COUT, 1], dt.float32)    # (0.5-b1)/g1
        nc.vector.tensor_tensor(rg2[:, :], rg1h[:, :], rgb[:, :], Alu.subtract)
        halfmb = const.tile([COUT, 1], dt.float32)  # 0.5 - scn_b
        nc.vector.tensor_scalar(halfmb[:, :], pars[:, 5:6], -1.0, 0.5,
                                Alu.mult, Alu.add)

        def bn_sums(tag, parts, fn=Act.Sqrt):
            """Combine AllReduced (mean, msq) partials -> mean, fn(var+eps)."""
            tot = const.tile([COUT, 2], dt.float32, tag=tag + "_tot")
            nc.vector.tensor_tensor(tot[:, :], parts[0], parts[1], Alu.add)
            if len(parts) > 2:
                nc.vector.tensor_tensor(tot[:, :], tot[:, :], parts[2], Alu.add)
            mean = tot[:, 0:1]
            var = const.tile([COUT, 1], dt.float32, tag=tag + "_var")
            nc.vector.tensor_tensor(var[:, :], mean, mean, Alu.mult)
            nc.vector.tensor_tensor(var[:, :], tot[:, 1:2], var[:, :],
                                    Alu.subtract)
            std = const.tile([COUT, 1], dt.float32, tag=tag + "_std")
            nc.scalar.activation(std[:, :], var[:, :], fn, bias=eps_t[:, :])
            return mean, std

        # ---- bn1 consts (vector stalls here on AR1b only) ----
        gs1a = const.tile([COUT, 4], dt.float32)
        nc.sync.dma_start(out=gs1a[:, :], in_=cc1ao[:, :])
        gs1b = const.tile([COUT, 2], dt.float32)
        nc.sync.dma_start(out=gs1b[:, :], in_=cc1bo[:, :])
        gs1c = const.tile([COUT, 2], dt.float32)
        nc.sync.dma_start(out=gs1c[:, :], in_=cc1co[:, :])

        m1, std1 = bn_sums("bn1", (gs1a[:, 0:2], gs1b[:, :]))
        # tau = 0.5*std1/g1 ; btil = b1*std1/g1 - mean1 ;
        # tau2 = tau - btil = (0.5-b1)*std1/g1 + mean1 (raw-space threshold,
        # computed first so spike(0) fires as early as possible)
        tau2 = const.tile([COUT, 1], dt.float32)
        nc.vector.scalar_tensor_tensor(tau2[:, :], std1[:, :], rg2[:, :],
                                       m1[:, :], Alu.mult, Alu.add)
        tau = const.tile([COUT, 1], dt.float32)
        nc.vector.tensor_tensor(tau[:, :], std1[:, :], rg1h[:, :], Alu.mult)
        btil = const.tile([COUT, 1], dt.float32)
        nc.vector.scalar_tensor_tensor(btil[:, :], std1[:, :], rgb[:, :],
                                       m1[:, :], Alu.mult, Alu.subtract)

        # ============ phase B: LIF recurrence + conv2 (fp16, partial) =======
        def fold(t):  # c' = cv1 + btil, in place, one fused (s-pair) slice
            sl = cv1f[:, t * SPT:(t + 1) * SPT]
            nc.scalar.activation(sl, sl, Act.Identity, bias=btil[:, :])

        cc2ai = dramp.tile([COUT, 2], dt.float32)
        cc2ao = dramp.tile([COUT, 2], dt.float32, addr_space="Shared")
        cc2bi = dramp.tile([COUT, 4], dt.float32)
        cc2bo = dramp.tile([COUT, 4], dt.float32, addr_space="Shared")
        ar2b = const.tile([COUT, 4], dt.float32, tag="ar2b")

        with tc.tile_pool(name="pu", bufs=2) as pu, \
             tc.tile_pool(name="pv", bufs=2) as pv, \
             tc.tile_pool(name="psp", bufs=3) as psp:

            def spike(v_ap, thr):  # contiguous fp16 {0,1} tile, both samples
                sq = psp.tile([COUT, SPT], dt.float16, tag="sq")
                nc.vector.tensor_scalar(sq[:, :], v_ap, thr[:, :], None,
                                        Alu.is_gt)
                return sq

            # spike(0) straight off raw cv1 (tau2), before fold(0) lands
            sq = spike(cv1f[:, 0:SPT], tau2)
            fold(0)
            v_prev = cv1f[:, 0:SPT]
            for t in range(T):
                if t + 1 < T:
                    fold(t + 1)
                    u = pu.tile([COUT, SPT], dt.float32, tag="u")
                    nc.vector.scalar_tensor_tensor(
                        u[:, :], v_prev, tau[:, :], v_prev, Alu.is_le, Alu.mult)
                    v = pv.tile([COUT, SPT], dt.float32, tag="v")
                    nc.vector.scalar_tensor_tensor(
                        v[:, :], u[:, :], float(d),
                        cv1f[:, (t + 1) * SPT:(t + 2) * SPT], Alu.mult, Alu.add)
                    v_prev = v[:, :]
                    sq_next = spike(v_prev, tau)
                else:
                    sq_next = None
                sqb = sq[:, 0]
                for s in range(BPC):
                    for c in range(2):
                        ps3 = psum.tile([COUT, CHUNK], dt.float32, tag="mm")
                        so = s * NPIX
                        oh0 = 14 * c
                        for ki, k in enumerate((4, 0, 1, 2, 3, 5, 6, 7, 8)):
                            kh, kw = divmod(k, 3)
                            r0 = oh0 + kh - 1
                            nr, o_r = 14, 0
                            if r0 < 0:          # kh=0, c=0
                                r0, nr, o_r = 0, 13, 1
                            elif r0 + 13 > 27:  # kh=2, c=1
                                nr = 13
                            if kw == 0:
                                cb, ncol, o_c = 0, WO - 1, 1
                            elif kw == 2:
                                cb, ncol, o_c = 1, WO - 1, 0
                            else:
                                cb, ncol, o_c = 0, WO, 0
                            outap = (ps3[:, :] if (nr == 14 and ncol == WO)
                                     else _ap(ps3[:, 0], o_r * WO + o_c,
                                              [[WO, nr], [1, ncol]]))
                            nc.tensor.matmul(
                                outap, w2[:, k, :],
                                _ap(sqb, so + r0 * WO + cb,
                                    [[WO, nr], [1, ncol]]),
                                start=(ki == 0), stop=(ki == 8),
                                skip_group_check=True)
                        off = (t * BPC + s) * NPIX + c * CHUNK
                        idx = 4 * t + 2 * s + c
                        nc.scalar.copy(cv2f[:, off:off + CHUNK], ps3[:, :])
                        nc.vector.bn_stats(out=st2[:, idx, :], in_=ps3[:, :])
                if t == 2:
                    # scn consts + scf threshold pass on the idle gpsimd
                    # (AR1c has landed by now; queues reach here late enough
                    # not to stall the LIF chain)
                    msc, stdsc = bn_sums("scn", (gs1a[:, 2:4], gs1c[:, :]))
                    rstds = const.tile([COUT, 1], dt.float32)
                    nc.vector.reciprocal(rstds[:, :], stdsc[:, :])
                    asc = const.tile([COUT, 1], dt.float32)
                    nc.vector.tensor_tensor(asc[:, :], pars[:, 4:5],
                                            rstds[:, :], Alu.mult)
                    nasc = const.tile([COUT, 1], dt.float32)
                    nc.vector.tensor_scalar_mul(nasc[:, :], asc[:, :], -1.0)
                    c1t = const.tile([COUT, 1], dt.float32)
                    nc.vector.scalar_tensor_tensor(
                        c1t[:, :], asc[:, :], msc[:, :], halfmb[:, :],
                        Alu.mult, Alu.add)
                if 3 <= t < 7:
                    # scf <- -asc*scf + c1t on SCALAR (gpsimd's version of
                    # this pass starves concurrent DVE SBUF access)
                    q0 = (t - 3) * (NLOC // 4)
                    sl = scf[:, q0:q0 + NLOC // 4]
                    nc.scalar.activation(sl, sl, Act.Identity,
                                         bias=c1t[:, :], scale=nasc[:, :])
                if t == A2SPLIT - 1:
                    # AR2a: bn2 partial, hidden under conv2 t=5,6,7
                    mv2a = const.tile([COUT, 2], dt.float32)
                    nc.vector.bn_aggr(out=mv2a[:, :],
                                      in_=st2[:, 0:4 * A2SPLIT, :])
                    ar2a = const.tile([COUT, 2], dt.float32)
                    sums_into(ar2a, mv2a, CNT_2A)
                    nc.sync.dma_start(out=cc2ai[:, :], in_=ar2a[:, :])
                    nc.gpsimd.collective_compute(
                        "AllReduce", Alu.add,
                        replica_groups=[list(range(NCORES))],
                        ins=[cc2ai[:, :].opt()], outs=[cc2ao[:, :].opt()])
                if t == T - 2:
                    # pre-aggregate t=5,6 for AR2b during t=7's compute, so
                    # only t=7's 4 chunks sit on the final trigger chain
                    mv2b1 = const.tile([COUT, 2], dt.float32)
                    nc.vector.bn_aggr(out=mv2b1[:, :],
                                      in_=st2[:, 4 * A2SPLIT:4 * (T - 1), :])
                    sums_into(ar2b[:, 0:2], mv2b1, 4 * (T - 1 - A2SPLIT) * CHUNK)
                sq = sq_next

        # ---- AR2b (bn2 remainder) - the only exposed tail collective ----
        mv2b = const.tile([COUT, 2], dt.float32)
        nc.vector.bn_aggr(out=mv2b[:, :], in_=st2[:, 4 * (T - 1):4 * T, :])
        sums_into(ar2b[:, 2:4], mv2b, 4 * CHUNK)
        nc.sync.dma_start(out=cc2bi[:, :], in_=ar2b[:, :])
        nc.gpsimd.collective_compute(
            "AllReduce", Alu.add, replica_groups=[list(range(NCORES))],
            ins=[cc2bi[:, :].opt()], outs=[cc2bo[:, :].opt()])

        gs2a = const.tile([COUT, 2], dt.float32)
        nc.sync.dma_start(out=gs2a[:, :], in_=cc2ao[:, :])
        gs2b = const.tile([COUT, 4], dt.float32)
        nc.sync.dma_start(out=gs2b[:, :], in_=cc2bo[:, :])

        m2v, std2 = bn_sums("bn2", (gs2a[:, :], gs2b[:, 0:2], gs2b[:, 2:4]))
        a2 = const.tile([COUT, 1], dt.float32)
        nc.vector.reciprocal(a2[:, :], std2[:, :])
        nc.vector.tensor_tensor(a2[:, :], a2[:, :], pars[:, 2:3], Alu.mult)
        b2 = const.tile([COUT, 1], dt.float32)
        nc.vector.tensor_tensor(b2[:, :], a2[:, :], m2v[:, :], Alu.mult)
        nc.vector.tensor_tensor(b2[:, :], pars[:, 3:4], b2[:, :], Alu.subtract)

        # epilogue: z' = a2*cv2 + b2 on scalar (z-rate 1.68us < cmp 1.78us,
        # so the vector cmp chain is the limiter either way; gpsimd versions
        # of either pass starve DVE SBUF access); compares on vector (the
        # only engine allowing fp32-in uint8-out cmp)
        with tc.tile_pool(name="outp", bufs=3) as op, \
             tc.tile_pool(name="zp", bufs=3) as zp:
            for t in range(T):
                off = t * SPT
                z = zp.tile([COUT, SPT], dt.float32, tag="z")
                nc.scalar.activation(z[:, :], cv2f[:, off:off + SPT],
                                     Act.Identity, bias=b2[:, :],
                                     scale=a2[:, :])
                ot = op.tile([COUT, SPT], dt.uint8, tag="ot")
                nc.vector.tensor_tensor(ot[:, :], z[:, :],
                                        scf[:, off:off + SPT], Alu.is_gt)
                for s in range(BPC):
                    nc.sync.dma_start(
                        out=out_d.ap()[s, :, t, :, :].rearrange("c h w -> c (h w)"),
                        in_=ot[:, s * NPIX:(s + 1) * NPIX])

    nc.compile()
    return nc


_CACHE = {}


def _bf16_hilo(a):
    import ml_dtypes
    a = np.asarray(a, np.float32)
    hi = a.astype(ml_dtypes.bfloat16)
    lo = (a - hi.astype(np.float32)).astype(ml_dtypes.bfloat16)
    return hi, lo


def _host_prep(inputs):
    xhi, xlo = _bf16_hilo(inputs["x"])
    xhi, xlo = np.ascontiguousarray(xhi), np.ascontiguousarray(xlo)
    w1t = np.ascontiguousarray(inputs["cv1_w"], np.float32).reshape(
        COUT, CIN, 3, 3).transpose(1, 2, 3, 0).reshape(CIN, 9, COUT)
    w1hi, w1lo = _bf16_hilo(w1t)
    w1h = np.ascontiguousarray(np.concatenate([w1hi, w1hi], axis=0))
    # paired lo weights: [Wlo(1,kw); Wlo(2,kw)] stacked on K
    w1p = np.ascontiguousarray(np.concatenate(
        [w1lo[:, 3:6, :], w1lo[:, 6:9, :]], axis=0))
    w1z = np.ascontiguousarray(w1lo[:, 0:3, :])
    w1c = np.ascontiguousarray(np.concatenate(
        [w1lo[:, 0, :], w1lo[:, 1, :]], axis=0))
    wst = np.asarray(inputs["sc_w"], np.float32).reshape(COUT, CIN).T
    wshi, wslo = _bf16_hilo(wst)
    wsh = np.ascontiguousarray(np.concatenate([wshi, wshi], axis=0))
    wsl = np.ascontiguousarray(np.concatenate([wslo, wslo], axis=0))
    w2t = np.ascontiguousarray(inputs["cv2_w"], np.float32).reshape(
        COUT, COUT, 3, 3).transpose(1, 2, 3, 0).reshape(COUT, 9, COUT)
    w2t = np.ascontiguousarray(w2t.astype(np.float16))
    pars = np.ascontiguousarray(np.stack(
        [np.asarray(inputs[p], np.float32).ravel()
         for p in ["bn1_g", "bn1_b", "bn2_g", "bn2_b", "scn_g", "scn_b"]],
        axis=1))
    d = float(1.0 / (1.0 + math.exp(-float(np.asarray(inputs["decay"]).ravel()[0]))))

    in_maps = []
    for c in range(NCORES):
        m = {"xhi": xhi[c * BPC:(c + 1) * BPC], "xlo": xlo[c * BPC:(c + 1) * BPC],
             "w1h": w1h, "w1p": w1p, "w1z": w1z, "w1c": w1c,
             "wsh": wsh, "wsl": wsl, "w2t": w2t, "pars": pars}
        in_maps.append(m)
    return in_maps, d


def kernel(**inputs):
    in_maps, d = _host_prep(inputs)
    key = round(d, 12)
    if key not in _CACHE:
        _CACHE[key] = build_nc(d)
    nc = _CACHE[key]

    res = run_bass_kernel_spmd(nc, in_maps, core_ids=list(range(NCORES)))
    out = np.concatenate([res.results[c]["out"] for c in range(NCORES)], axis=0)
    return np.ascontiguousarray(out, dtype=np.float32)
